# revision 21
# baseline (speedup 1.0000x reference)
"""MoE (E=8 experts, top-2, D=1024, T=8192) — expert-parallel Trainium2 kernel.

Strategy (per the expert-parallel sharding hint):
  - Host computes the gate (0.1% of FLOPs) and shards tokens: each of the 8
    NeuronCores owns one expert and receives exactly the tokens routed to it
    (padded to a common capacity C, multiple of 64).
  - Each core runs the dense expert FFN + layernorm + combine-weight scaling
    over its routed tokens (99.9% of the FLOPs).
  - Host gathers the per-expert outputs back into token order and sums the
    K=2 contributions.

Both FFN matmuls run in fp8e4m3 with perf_mode=DoubleRow (two contraction
rows per PE cell -> ~1.8x the bf16 matmul rate). Max rel err vs the fp32
reference is 1.68e-2 (measured; deterministic), inside the 2e-2 gate. Both
weight matrices are fp8-resident in SBUF (2 MB each) so there is no per-tile
weight streaming; per-tile DMA is just the token activations in fp8 (mm1
operand) + bf16 (residual), the combine weights, and the bf16 output.

Every DRAM tensor is host-packed into the exact SBUF layout (weights
[P, m, ko, 128] m-major; activations tile-blocked [P, ko, ct]) so each DMA is
one contiguous run per partition (128 descriptors) — strided layouts cost
1-8us of DGE descriptor generation per DMA and serialized the startup.
Weight DMAs ride the SP ring; activation DMAs ride the ACT ring so gelus
never queue behind weight descriptor generation.

LN statistics: z chunks (bf16) are pre-summed across the 8 feature chunks on
the DVE (squared on ACT — `square` exists in every ACT table set), so the
partition reduction is a single bf16 ones-matmul per statistic; the last tile
instead accumulates all 8 chunks on the (then idle) PE to keep the drain
phase off the DVE. rw = wv * AbsReciprocalSqrt(var+eps) — a single-set ACT
function (Gelu/Sqrt/Ln/Exp mixes pay 1.3us table reloads per switch), emitted
after the full mm1 so the one table switch per tile never blocks gelus the
next mm2 needs. The final two capacity tiles are split evenly (the drain is
DVE-serial and scales with the last tiles' sizes), and ~3.4us of dummy
matmuls on constants during the startup DMA wait pre-warm the PE's HAM clock
gate (otherwise the first real matmuls run at 1.2 GHz).
"""

import sys

sys.path.insert(0, "/opt/trn_rl_repo")

import ml_dtypes
import numpy as np

E, K, D = 8, 2, 1024
H = 2 * D
B, S = 2, 4096
T = B * S
CT = 512  # max token tile (fp8 DR moving free = 2*CT = 1024 max)
PAD = 64  # capacity padding granularity (keeps all strides %16 == 0)
P = 128
KO1 = D // P  # 8
MO1 = H // P  # 16
KO2 = H // P  # 16
MO2 = D // P  # 8

_kernel_cache = {}


def _tile_sizes(C):
    """First tile 448 (fast startup), full 512s, remainder split into two
    roughly equal tiles (the drain phase scales with the last tiles)."""
    sizes = []
    rest = C
    if rest > CT:
        sizes.append(448)
        rest -= 448
    while rest > 768:
        sizes.append(CT)
        rest -= CT
    if rest > 384:
        a = (rest // 2 + PAD - 1) // PAD * PAD
        sizes += [a, rest - a]
    elif rest:
        sizes.append(rest)
    tiles = []
    t0 = 0
    for ct in sizes:
        tiles.append((t0, ct))
        t0 += ct
    return tiles


def _build_bass(C, use_b2, use_lng, use_lnb):
    """Per-core expert-FFN kernel for capacity C (multiple of PAD)."""
    import concourse.tile as tile
    from concourse import bacc, bass, mybir

    f32 = mybir.dt.float32
    bf16 = mybir.dt.bfloat16
    f8 = mybir.dt.float8e4
    AF = mybir.ActivationFunctionType
    OP = mybir.AluOpType
    DR = mybir.MatmulPerfMode.DoubleRow

    assert C % PAD == 0
    tiles = _tile_sizes(C)

    nc = bacc.Bacc("TRN2", target_bir_lowering=False, debug=False)
    # all inputs host-packed to the exact SBUF layout (fully contiguous DMAs)
    x8_d = nc.dram_tensor("x8", [P, KO1 * C], f8, kind="ExternalInput")
    xb_d = nc.dram_tensor("xb", [P, MO2 * C], bf16, kind="ExternalInput")
    w1_d = nc.dram_tensor("w1", [P, MO1, KO1, P], f8, kind="ExternalInput")
    w2_d = nc.dram_tensor("w2", [P, MO2, KO2, P], f8, kind="ExternalInput")
    b1_d = nc.dram_tensor("b1", [P, MO1], f32, kind="ExternalInput")
    if use_b2:
        b2_d = nc.dram_tensor("b2", [P, MO2], f32, kind="ExternalInput")
    if use_lng:
        lng_d = nc.dram_tensor("ln_g", [P, MO2], f32, kind="ExternalInput")
    if use_lnb:
        lnb_d = nc.dram_tensor("ln_b", [P, MO2], f32, kind="ExternalInput")
    wv_d = nc.dram_tensor("wv", [P, C], f32, kind="ExternalInput")
    out_d = nc.dram_tensor("outT", [P, MO2 * C], bf16, kind="ExternalOutput")

    with tile.TileContext(nc) as tc:
        with (
            tc.tile_pool(name="singles", bufs=1) as singles,
            tc.tile_pool(name="xp", bufs=3) as xp,
            tc.tile_pool(name="xbp", bufs=3) as xbp,
            tc.tile_pool(name="hp", bufs=2) as hp,
            tc.tile_pool(name="zp", bufs=2) as zp,
            tc.tile_pool(name="zqp", bufs=2) as zqp,
            tc.tile_pool(name="accp", bufs=2) as accp,
            tc.tile_pool(name="wvp", bufs=2) as wvp,
            tc.tile_pool(name="stp", bufs=2) as stp,
            tc.tile_pool(name="ocp", bufs=2) as ocp,
            tc.tile_pool(name="psmm", bufs=6, space="PSUM") as psmm,
            tc.tile_pool(name="psst", bufs=1, space="PSUM") as psst,
        ):
            ct0 = tiles[0][1]

            def x_tile(i):
                t0, ct = tiles[i]
                xt = xp.tile([P, KO1, CT], f8, name="x8_sb")
                nc.scalar.dma_start(
                    xt[:, :, :ct], x8_d[:, KO1 * t0 : KO1 * (t0 + ct)]
                )
                bt = xbp.tile([P, MO2, CT], bf16, name="xb_sb")
                nc.scalar.dma_start(
                    bt[:, :, :ct], xb_d[:, MO2 * t0 : MO2 * (t0 + ct)]
                )
                return (xt, bt)

            # ---- startup: tile 0's activations (ACT ring) + weights (SP) ----
            tok_tiles = {0: x_tile(0)}
            # w1 in 4 m-major quarters so tile 0's first m-groups start early
            w1_sb = singles.tile([P, MO1, KO1, P], f8)
            for q in range(4):
                nc.sync.dma_start(
                    w1_sb[:, 4 * q : 4 * (q + 1)], w1_d[:, 4 * q : 4 * (q + 1)]
                )
            ones_bf = singles.tile([P, P], bf16)
            nc.vector.memset(ones_bf[:], 1.0)
            eps_sb = singles.tile([P, 1], f32)
            nc.vector.memset(eps_sb[:], 1e-6)
            b1_sb = singles.tile([P, MO1], f32)
            nc.scalar.dma_start(b1_sb[:], b1_d[:])
            # warm the PE's HAM clock gate during the startup DMA wait
            warm_bf = singles.tile([P, CT], bf16)
            nc.vector.memset(warm_bf[:], 0.0)
            for _ in range(8):
                ps_w = psmm.tile([P, CT], f32, name="ps_w", tag="mm")
                nc.tensor.matmul(ps_w[:], ones_bf[:], warm_bf[:])
            if use_b2:
                b2_sb = singles.tile([P, MO2], f32)
                nc.scalar.dma_start(b2_sb[:], b2_d[:])
            if use_lng:
                lng_sb = singles.tile([P, MO2], f32)
                nc.scalar.dma_start(lng_sb[:], lng_d[:])
            if use_lnb:
                lnb_sb = singles.tile([P, MO2], f32)
                nc.scalar.dma_start(lnb_sb[:], lnb_d[:])
            # w2 resident in two mo-halves (mm2 groups 0-3 need only half 1)
            w2_sb = singles.tile([P, MO2, KO2, P], f8)
            nc.sync.dma_start(w2_sb[:, : MO2 // 2], w2_d[:, : MO2 // 2])
            nc.sync.dma_start(w2_sb[:, MO2 // 2 :], w2_d[:, MO2 // 2 :])

            def emit_mm1(t, ct, prev_st):
                """mm1 + gelu into a fresh fp8 hT tile; the previous tile's
                mm2 groups interleave 1:2 at even m-groups (the last one ends
                an mm1-group before the stats matmuls, hiding DVE latency)."""
                hT_sb = hp.tile([P, KO2, CT], f8, name="hT_sb")
                x8_sb = tok_tiles[t][0]
                for m in range(MO1):
                    ps = psmm.tile([P, CT], f32, name="ps_mm", tag="mm")
                    for kk in range(KO1 // 2):
                        nc.tensor.matmul(
                            ps[:, :ct],
                            w1_sb[:, m, 2 * kk : 2 * kk + 2],
                            x8_sb[:, 2 * kk : 2 * kk + 2, :ct],
                            start=(kk == 0),
                            stop=(kk == KO1 // 2 - 1),
                            perf_mode=DR,
                        )
                    nc.scalar.activation(
                        out=hT_sb[:, m, :ct],
                        in_=ps[:, :ct],
                        func=AF.Gelu,
                        bias=b1_sb[:, m : m + 1],
                        scale=1.0,
                    )
                    if prev_st is not None and m % 2 == 0:
                        mm2_group(prev_st, m // 2)
                return hT_sb

            def mm2_begin(t, t0, ct, hT_sb, last=False):
                wv_sb = wvp.tile([P, CT], f32, name="wv_sb")[:, :ct]
                nc.sync.dma_start(wv_sb, wv_d[:, t0 : t0 + ct])
                st = {
                    "t": t, "t0": t0, "ct": ct, "hT_sb": hT_sb,
                    "wv_sb": wv_sb, "last": last,
                    "xb_sb": tok_tiles[t][1],
                    "z_sb": zp.tile([P, MO2, CT], bf16, name="z_sb"),
                }
                if last:
                    # drain phase computes stats with PE matmuls (PE is idle
                    # there; the DVE chain is the critical path), so keep the
                    # squared chunks instead of accumulating on the DVE
                    st["zq_all"] = zqp.tile(
                        [P, MO2, CT], bf16, name="zq_all", tag="zqall", bufs=1
                    )
                else:
                    st["sacc"] = accp.tile([P, CT], bf16, name="sacc", tag="sacc")
                    st["qacc"] = accp.tile([P, CT], bf16, name="qacc", tag="qacc")
                return st

            def mm2_group(st, mo):
                ct = st["ct"]
                ps = psmm.tile([P, CT], f32, name="ps_mm", tag="mm")
                for jj in range(KO2 // 2):
                    nc.tensor.matmul(
                        ps[:, :ct],
                        w2_sb[:, mo, 2 * jj : 2 * jj + 2],
                        st["hT_sb"][:, 2 * jj : 2 * jj + 2, :ct],
                        start=(jj == 0),
                        stop=(jj == KO2 // 2 - 1),
                        perf_mode=DR,
                    )
                zc = st["z_sb"][:, mo, :ct]
                nc.vector.tensor_tensor(
                    zc, ps[:, :ct], st["xb_sb"][:, mo, :ct], OP.add
                )
                if use_b2:
                    nc.vector.tensor_scalar_add(zc, zc, b2_sb[:, mo : mo + 1])
                if st["last"]:
                    nc.scalar.activation(
                        out=st["zq_all"][:, mo, :ct], in_=zc,
                        func=AF.Square, scale=1.0,
                    )
                    return
                zq = zqp.tile([P, CT], bf16, name="zq")[:, :ct]
                nc.scalar.activation(out=zq, in_=zc, func=AF.Square, scale=1.0)
                sacc, qacc = st["sacc"][:, :ct], st["qacc"][:, :ct]
                if mo == 0:
                    nc.vector.tensor_copy(sacc, zc)
                    nc.vector.tensor_copy(qacc, zq)
                else:
                    nc.vector.tensor_tensor(sacc, sacc, zc, OP.add)
                    nc.vector.tensor_tensor(qacc, qacc, zq, OP.add)

            def ln_tail(st):
                ct, t0, wv_sb = st["ct"], st["t0"], st["wv_sb"]
                z_sb = st["z_sb"]
                # partition reduction + broadcast: one bf16 ones-matmul per
                # stat over the DVE-presummed chunks; the last tile skips the
                # presums and accumulates all 8 chunks on the (idle) PE
                ps_S = psst.tile([P, CT], f32, name="ps_S", tag="psS")
                ps_Q = psst.tile([P, CT], f32, name="ps_Q", tag="psQ")
                if st["last"]:
                    for mo in range(MO2):
                        nc.tensor.matmul(
                            ps_S[:, :ct], ones_bf[:], z_sb[:, mo, :ct],
                            start=(mo == 0), stop=(mo == MO2 - 1),
                        )
                    for mo in range(MO2):
                        nc.tensor.matmul(
                            ps_Q[:, :ct], ones_bf[:], st["zq_all"][:, mo, :ct],
                            start=(mo == 0), stop=(mo == MO2 - 1),
                        )
                else:
                    nc.tensor.matmul(ps_S[:, :ct], ones_bf[:], st["sacc"][:, :ct])
                    nc.tensor.matmul(ps_Q[:, :ct], ones_bf[:], st["qacc"][:, :ct])
                mean32 = stp.tile([P, CT], f32, name="mean32", tag="mean32")[:, :ct]
                nc.vector.tensor_scalar_mul(mean32, ps_S[:, :ct], 1.0 / D)
                var32 = stp.tile([P, CT], f32, name="var32", tag="var32")[:, :ct]
                nc.vector.tensor_scalar_mul(var32, ps_Q[:, :ct], 1.0 / D)
                msq = stp.tile([P, CT], f32, name="msq", tag="msq")[:, :ct]
                nc.vector.tensor_tensor(msq, mean32, mean32, OP.mult)
                nc.vector.tensor_tensor(var32, var32, msq, OP.subtract)
                # rw = wv * rsqrt(var + eps); Abs_reciprocal_sqrt is a single
                # ACT function in a single table set (var+eps > 0)
                rs = stp.tile([P, CT], f32, name="rs", tag="rs")[:, :ct]
                nc.scalar.activation(
                    out=rs, in_=var32, func=AF.Abs_reciprocal_sqrt,
                    bias=eps_sb[:], scale=1.0,
                )
                nc.vector.tensor_tensor(rs, rs, wv_sb, OP.mult)
                rwb = stp.tile([P, CT], bf16, name="rwb", tag="rwb")[:, :ct]
                nc.vector.tensor_copy(rwb, rs)
                meanb = stp.tile([P, CT], bf16, name="meanb", tag="meanb")[:, :ct]
                nc.vector.tensor_copy(meanb, mean32)
                # normalize + scale; two whole-tile broadcast DVE ops, one
                # contiguous store
                oc = ocp.tile([P, MO2, CT], bf16, name="oc")
                if not (use_lng or use_lnb):
                    a0, a1 = bass.broadcast_tensor_aps(
                        z_sb[:, :, :ct], meanb[:, None, :]
                    )
                    nc.vector.tensor_tensor(oc[:, :, :ct], a0, a1, OP.subtract)
                    b0, b1_ = bass.broadcast_tensor_aps(
                        oc[:, :, :ct], rwb[:, None, :]
                    )
                    nc.vector.tensor_tensor(oc[:, :, :ct], b0, b1_, OP.mult)
                else:
                    for mo in range(MO2):
                        d = oc[:, mo, :ct]
                        nc.vector.tensor_tensor(
                            d, z_sb[:, mo, :ct], meanb, OP.subtract
                        )
                        nc.vector.tensor_tensor(d, d, rwb, OP.mult)
                        if use_lng:
                            nc.vector.tensor_scalar_mul(d, d, lng_sb[:, mo : mo + 1])
                        if use_lnb:
                            lb = stp.tile([P, CT], f32, name="lb", tag="lb")[:, :ct]
                            nc.vector.tensor_scalar_mul(lb, wv_sb, lnb_sb[:, mo : mo + 1])
                            nc.vector.tensor_tensor(d, d, lb, OP.add)
                nc.sync.dma_start(
                    out_d[:, MO2 * t0 : MO2 * (t0 + ct)], oc[:, :, :ct]
                )

            prev_st = None
            penult_st = None
            last = len(tiles) - 1
            for t, (t0, ct) in enumerate(tiles):
                hT_sb = emit_mm1(t, ct, prev_st)
                if prev_st is not None:
                    if t < last:
                        # after the full mm1 so the ACT table switch for the
                        # rsqrt never sits between gelus the next mm2 needs
                        ln_tail(prev_st)
                    else:
                        # deferred past the last tile's mm2 groups so their
                        # z-adds aren't stuck behind this normalize in the
                        # DVE FIFO
                        penult_st = prev_st
                # prefetch token tiles; tile 2's fetch is deferred to t=1 to
                # keep startup HBM bandwidth for the weights
                for tn_i in ([1] if t == 0 else ([2, 3] if t == 1 else [t + 2])):
                    if tn_i <= last:
                        tok_tiles[tn_i] = x_tile(tn_i)
                prev_st = mm2_begin(t, t0, ct, hT_sb, last=(t == last))
            for mo in range(MO2):
                mm2_group(prev_st, mo)
            if penult_st is not None:
                ln_tail(penult_st)
            ln_tail(prev_st)

    nc.finalize()
    return nc


def _route(x, gate_w):
    """Host gate: top-2 per token + softmax combine weights (matches
    jax.lax.top_k tie-breaking: lower index wins)."""
    xt = x.reshape(-1, D)
    scores = xt.astype(np.float32) @ gate_w.astype(np.float32)  # [T, E]
    e0 = np.argmax(scores, axis=1)
    s0 = scores[np.arange(T), e0]
    masked = scores.copy()
    masked[np.arange(T), e0] = -np.inf
    e1 = np.argmax(masked, axis=1)
    s1 = masked[np.arange(T), e1]
    mx = np.maximum(s0, s1)
    z0 = np.exp((s0 - mx).astype(np.float64))
    z1 = np.exp((s1 - mx).astype(np.float64))
    den = z0 + z1
    w0 = (z0 / den).astype(np.float32)
    w1 = (z1 / den).astype(np.float32)
    return xt, e0, e1, w0, w1


def _pack_tokens(xT, tiles, ko, dtype):
    """[D, C] -> [P, ko*C] tile-blocked: per tile, [ko, P, ct] -> [P, ko*ct]
    contiguous per partition (one DMA descriptor per partition per tile)."""
    C = xT.shape[1]
    out = np.empty((P, ko * C), dtype)
    for t0, ct in tiles:
        blk = xT[:, t0 : t0 + ct].reshape(ko, P, ct)
        out[:, ko * t0 : ko * (t0 + ct)] = (
            blk.transpose(1, 0, 2).reshape(P, ko * ct)
        )
    return out


def _pack_w(w, ko, mo, dtype):
    """[K, M] -> [P, mo, ko, P] m-major (the per-matmul lhsT [P, 2, 128]
    slices are contiguous, and quarter-loads by m arrive in consumption
    order)."""
    t = np.asarray(w, np.float32).reshape(ko, P, mo, P)
    return np.ascontiguousarray(t.transpose(1, 2, 0, 3)).astype(dtype)


def kernel(x, gate_w, w1, b1, w2, b2, ln_g, ln_b):
    from concourse.bass_utils import run_bass_kernel_spmd

    x = np.asarray(x)
    xt, e0, e1, wk0, wk1 = _route(x, np.asarray(gate_w))

    # slot assignment: expert e's token list = tokens with e0==e, then e1==e
    idx_e, wv_e = [], []
    for e in range(E):
        i0 = np.nonzero(e0 == e)[0]
        i1 = np.nonzero(e1 == e)[0]
        idx_e.append(np.concatenate([i0, i1]))
        wv_e.append(np.concatenate([wk0[i0], wk1[i1]]))
    maxn = max(len(i) for i in idx_e)
    C = max(PAD, -(-maxn // PAD) * PAD)
    tiles = _tile_sizes(C)

    use_b2 = bool(np.any(np.asarray(b2) != 0))
    use_lng = bool(np.any(np.asarray(ln_g) != 1))
    use_lnb = bool(np.any(np.asarray(ln_b) != 0))
    key = (C, use_b2, use_lng, use_lnb)
    if key not in _kernel_cache:
        _kernel_cache[key] = _build_bass(C, use_b2, use_lng, use_lnb)
    nc = _kernel_cache[key]

    f8 = ml_dtypes.float8_e4m3
    bf = ml_dtypes.bfloat16

    def chunked(a, n):  # [n*P] -> [P, n] host prelayout
        return np.ascontiguousarray(np.asarray(a, np.float32).reshape(n, P).T)

    in_maps = []
    for e in range(E):
        n = len(idx_e[e])
        xTe = np.zeros((D, C), np.float32)
        xTe[:, :n] = xt[idx_e[e]].T
        wve = np.zeros((C,), np.float32)
        wve[:n] = wv_e[e]
        im = {
            "x8": _pack_tokens(xTe, tiles, KO1, f8),
            "xb": _pack_tokens(xTe, tiles, MO2, bf),
            "w1": _pack_w(np.asarray(w1)[e], KO1, MO1, f8),
            "w2": _pack_w(np.asarray(w2)[e], KO2, MO2, f8),
            "b1": chunked(np.asarray(b1)[e], MO1),
            "wv": np.broadcast_to(wve, (P, C)).copy(),
        }
        if use_b2:
            im["b2"] = chunked(np.asarray(b2)[e], MO2)
        if use_lng:
            im["ln_g"] = chunked(np.asarray(ln_g)[e], MO2)
        if use_lnb:
            im["ln_b"] = chunked(np.asarray(ln_b)[e], MO2)
        in_maps.append(im)

    res = run_bass_kernel_spmd(nc, in_maps, core_ids=list(range(E)))
    kernel.last_results = res

    # unpack the tile-blocked outputs back to [E, D, C]
    Y = np.empty((E, D, C), np.float32)
    for e in range(E):
        o = res.results[e]["outT"]  # [P, MO2*C] tile-blocked bf16
        for t0, ct in tiles:
            blk = o[:, MO2 * t0 : MO2 * (t0 + ct)].reshape(P, MO2, ct)
            Y[e, :, t0 : t0 + ct] = (
                blk.transpose(1, 0, 2).reshape(D, ct).astype(np.float32)
            )

    # combine: token t's two contributions live at known (expert, slot) pairs
    slot0 = np.empty(T, np.int64)
    slot1 = np.empty(T, np.int64)
    for e in range(E):
        n0 = int(np.sum(e0 == e))
        slot0[e0 == e] = np.arange(n0)
        slot1[e1 == e] = n0 + np.arange(int(np.sum(e1 == e)))
    out = Y[e0, :, slot0] + Y[e1, :, slot1]  # [T, D]
    return out.reshape(x.shape).astype(np.float32)
